# revision 4
# baseline (speedup 1.0000x reference)
import os, sys
import numpy as np

sys.path.insert(0, "/opt/trn_rl_repo")
import concourse.bass as bass
import concourse.bacc as bacc
import concourse.mybir as mybir
from concourse.tile import TileContext
from concourse.bass_utils import run_bass_kernel_spmd

F32 = mybir.dt.float32
AF = mybir.ActivationFunctionType
ALU = mybir.AluOpType

B, T, D, L, H = 64, 512, 400, 25, 200
NC = 8
BC = B // NC  # 8 sequences per core
HC = H // 2   # 100, hidden chunk (partition dim)
G4 = 4 * H    # 800 gate width per direction

_GRAPH = None
LAST_EXEC_NS = None


def _build_graph():
    nc = bacc.Bacc("TRN2", target_bir_lowering=False, debug=False, num_devices=NC)
    x = nc.dram_tensor("x", [BC * T, D], F32, kind="ExternalInput")
    wihT = nc.dram_tensor("wihT", [D, 2 * G4], F32, kind="ExternalInput")
    whhT = nc.dram_tensor("whhT", [H, 2 * G4], F32, kind="ExternalInput")
    wlT = nc.dram_tensor("wlT", [2 * H, L], F32, kind="ExternalInput")
    bias_g = nc.dram_tensor("bias_g", [HC, 16], F32, kind="ExternalInput")
    bl = nc.dram_tensor("bl", [L, 1], F32, kind="ExternalInput")
    mask40 = nc.dram_tensor("mask40", [40, T], F32, kind="ExternalInput")
    ident = nc.dram_tensor("ident", [128, 128], F32, kind="ExternalInput")
    eT = nc.dram_tensor("eT", [L, T * BC], F32, kind="ExternalOutput")
    xw = nc.dram_tensor("xw", [T, 2 * BC, G4], F32)  # scratch [t, (d,b), (g,h')]

    with TileContext(nc) as tc:
        with (
            tc.tile_pool(name="const", bufs=1) as cp,
            tc.tile_pool(name="psc", bufs=2, space="PSUM") as psc,
        ):
            ident_sb = cp.tile([128, 128], F32)
            nc.sync.dma_start(out=ident_sb[:], in_=ident[:])
            whh_sb = cp.tile([HC, 2 * 2 * G4], F32)  # chunk c at c*1600
            for c in range(2):
                nc.sync.dma_start(
                    out=whh_sb[:, c * 2 * G4 : (c + 1) * 2 * G4],
                    in_=whhT[c * HC : (c + 1) * HC, :],
                )
            mask_sb = cp.tile([40, T], F32)
            nc.sync.dma_start(out=mask_sb[:], in_=mask40[:])
            bias_sb = cp.tile([HC, 16], F32)
            nc.sync.dma_start(out=bias_sb[:], in_=bias_g[:])
            bl_sb = cp.tile([L, 1], F32)
            nc.sync.dma_start(out=bl_sb[:], in_=bl[:])
            wl_sb = cp.tile([HC, 4 * L], F32)
            for kc in range(4):
                nc.sync.dma_start(
                    out=wl_sb[:, kc * L : (kc + 1) * L],
                    in_=wlT[kc * HC : (kc + 1) * HC, :],
                )
            ztile = cp.tile([HC, BC], F32)
            nc.vector.memset(ztile[:], 0.0)
            # h history, transposed layout: col = ((d*2+c)*T + t)*BC + b
            h_hist = cp.tile([HC, 2 * 2 * T * BC], F32)
            c40 = cp.tile([40, H], F32)
            h40 = cp.tile([40, H], F32)
            nc.vector.memset(c40[:], 0.0)
            nc.vector.memset(h40[:], 0.0)

            # ---- Phase 1: transpose X -> XT [100, (k, b*T)] ----
            with (
                tc.tile_pool(name="ph1", bufs=3) as p1,
                tc.tile_pool(name="xtp", bufs=1) as xp,
                tc.tile_pool(name="ps1", bufs=4, space="PSUM") as pp1,
            ):
                XT = xp.tile([HC, 4 * BC * T], F32)
                nrow = (BC * T) // 128  # 32
                for r in range(nrow):
                    xrow = p1.tile([128, D], F32, tag="xrow")
                    nc.sync.dma_start(out=xrow[:], in_=x[r * 128 : (r + 1) * 128, :])
                    for k in range(4):
                        ps = pp1.tile([HC, 128], F32, tag="tps1")
                        nc.tensor.transpose(
                            ps[:], xrow[:, k * HC : (k + 1) * HC], ident_sb[:]
                        )
                        nc.scalar.activation(
                            XT[:, k * BC * T + r * 128 : k * BC * T + (r + 1) * 128],
                            ps[:],
                            AF.Copy,
                        )

                # ---- Phase 2: input GEMM -> xw scratch ----
                with (
                    tc.tile_pool(name="ph2", bufs=3) as p2,
                    tc.tile_pool(name="ps2", bufs=4, space="PSUM") as pp2,
                ):
                    wih_sb = []
                    for k in range(4):
                        wt = xp.tile([HC, 2 * G4], F32, tag=f"wih{k}")
                        nc.sync.dma_start(
                            out=wt[:], in_=wihT[k * HC : (k + 1) * HC, :]
                        )
                        wih_sb.append(wt)
                    xw_r = xw[:].rearrange("t r c -> r c t")  # [(d,b), (g,h'), t]
                    for m in range(16):  # (d, g, hh)
                        d_, g_, hh = m // 8, (m // 2) % 4, m % 2
                        for b8 in range(BC):
                            ps = pp2.tile([HC, T], F32, tag="ps2t")
                            for k in range(4):
                                nc.tensor.matmul(
                                    ps[:],
                                    wih_sb[k][:, m * HC : (m + 1) * HC],
                                    XT[:, k * BC * T + b8 * T : k * BC * T + (b8 + 1) * T],
                                    start=(k == 0),
                                    stop=(k == 3),
                                )
                            tmp = p2.tile([HC, T], F32, tag="tmp2")
                            nc.scalar.activation(
                                tmp[:], ps[:], AF.Identity, bias=bias_sb[:, m : m + 1]
                            )
                            col0 = g_ * H + hh * HC
                            nc.sync.dma_start(
                                out=xw_r[d_ * BC + b8, col0 : col0 + HC, :],
                                in_=tmp[:],
                            )

            # ---- Phase 3: BiLSTM scan ----
            with (
                tc.tile_pool(name="ph3", bufs=3) as p3,
                tc.tile_pool(name="ps3", bufs=2, space="PSUM") as pp3,
            ):
                for t in range(T):
                    tb = T - 1 - t  # backward time index
                    xwt = p3.tile([40, G4], F32, tag="xwt")
                    nc.sync.dma_start(out=xwt[0:8, :], in_=xw[t, 0:BC, :])
                    nc.sync.dma_start(out=xwt[32:40, :], in_=xw[tb, BC : 2 * BC, :])
                    gps0 = pp3.tile([40, 400], F32, tag="gp0")
                    gps1 = pp3.tile([40, 400], F32, tag="gp1")
                    for d_ in range(2):
                        tt = t if d_ == 0 else tb
                        tprev = tt - 1 if d_ == 0 else tt + 1
                        for c in range(2):
                            if t == 0:
                                lhsT = ztile[:]
                            else:
                                off = ((d_ * 2 + c) * T + tprev) * BC
                                lhsT = h_hist[:, off : off + BC]
                            for nh in range(2):
                                dst = (gps0 if nh == 0 else gps1)[
                                    d_ * 32 : d_ * 32 + 8, :
                                ]
                                rhs = whh_sb[
                                    :,
                                    c * 2 * G4 + d_ * G4 + nh * 400 : c * 2 * G4
                                    + d_ * G4
                                    + (nh + 1) * 400,
                                ]
                                nc.tensor.matmul(
                                    dst, lhsT, rhs, start=(c == 0), stop=(c == 1)
                                )
                    g0 = p3.tile([40, 400], F32, tag="g0s")
                    g1 = p3.tile([40, 400], F32, tag="g1s")
                    nc.vector.tensor_add(g0[:], gps0[:], xwt[:, 0:400])
                    nc.vector.tensor_add(g1[:], gps1[:], xwt[:, 400:800])
                    s0 = p3.tile([40, 400], F32, tag="s0")
                    th = p3.tile([40, 400], F32, tag="th")
                    nc.scalar.activation(s0[:], g0[:], AF.Sigmoid)  # i | f
                    nc.scalar.activation(th[:, 0:H], g1[:, 0:H], AF.Tanh)  # g~
                    nc.scalar.activation(th[:, H:400], g1[:, H:400], AF.Sigmoid)  # o
                    t1 = p3.tile([40, H], F32, tag="t1")
                    t2 = p3.tile([40, H], F32, tag="t2")
                    cn = p3.tile([40, H], F32, tag="cn")
                    nc.vector.tensor_mul(t1[:], s0[:, 0:H], th[:, 0:H])
                    nc.vector.tensor_mul(t2[:], s0[:, H:400], c40[:])
                    nc.vector.tensor_add(cn[:], t1[:], t2[:])
                    cd = p3.tile([40, H], F32, tag="cd")
                    nc.vector.tensor_sub(cd[:], cn[:], c40[:])
                    nc.vector.scalar_tensor_tensor(
                        c40[:], cd[:], mask_sb[:, t : t + 1], c40[:],
                        op0=ALU.mult, op1=ALU.add,
                    )
                    thc = p3.tile([40, H], F32, tag="thc")
                    nc.scalar.activation(thc[:], c40[:], AF.Tanh)
                    hn = p3.tile([40, H], F32, tag="hn")
                    nc.vector.tensor_mul(hn[:], th[:, H:400], thc[:])
                    hd = p3.tile([40, H], F32, tag="hd")
                    nc.vector.tensor_sub(hd[:], hn[:], h40[:])
                    nc.vector.scalar_tensor_tensor(
                        h40[:], hd[:], mask_sb[:, t : t + 1], h40[:],
                        op0=ALU.mult, op1=ALU.add,
                    )
                    for d_ in range(2):
                        tt = t if d_ == 0 else tb
                        for c in range(2):
                            tps = pp3.tile([HC, BC], F32, tag="tp3")
                            nc.tensor.transpose(
                                tps[:],
                                h40[d_ * 32 : d_ * 32 + 8, c * HC : (c + 1) * HC],
                                ident_sb[d_ * 32 : d_ * 32 + 8, d_ * 32 : d_ * 32 + 8],
                            )
                            off = ((d_ * 2 + c) * T + tt) * BC
                            nc.scalar.activation(
                                h_hist[:, off : off + BC], tps[:], AF.Copy
                            )

            # ---- Phase 4: emissions GEMM -> eT ----
            with (
                tc.tile_pool(name="ph4", bufs=3) as p4,
                tc.tile_pool(name="ps4", bufs=4, space="PSUM") as pp4,
            ):
                for n in range(8):
                    ps = pp4.tile([L, T], F32, tag="ps4t")
                    for kc in range(4):
                        rhs = h_hist[:, kc * T * BC + n * T : kc * T * BC + (n + 1) * T]
                        nc.tensor.matmul(
                            ps[:],
                            wl_sb[:, kc * L : (kc + 1) * L],
                            rhs,
                            start=(kc == 0),
                            stop=(kc == 3),
                        )
                    et = p4.tile([L, T], F32, tag="et4")
                    nc.scalar.activation(et[:], ps[:], AF.Identity, bias=bl_sb[:])
                    nc.sync.dma_start(out=eT[:, n * T : (n + 1) * T], in_=et[:])

    nc.compile()
    return nc


def _logsumexp(a, axis):
    m = a.max(axis=axis, keepdims=True)
    return (m + np.log(np.exp(a - m).sum(axis=axis, keepdims=True))).squeeze(axis)


def _crf_llh_np(e, labels, mask, trans, start, end):
    em = np.take_along_axis(e, labels[..., None], axis=-1)[..., 0]
    tr = trans[labels[:, :-1], labels[:, 1:]]
    last = (mask.sum(1) - 1.0).astype(np.int32)
    y_last = np.take_along_axis(labels, last[:, None], axis=1)[:, 0]
    num = (
        start[labels[:, 0]]
        + em[:, 0]
        + (mask[:, 1:] * (em[:, 1:] + tr)).sum(1)
        + end[y_last]
    )
    alpha = start[None, :] + e[:, 0]
    for t in range(1, e.shape[1]):
        cand = alpha[:, :, None] + trans[None]
        m = cand.max(1)
        new = m + np.log(np.exp(cand - m[:, None, :]).sum(1)) + e[:, t]
        alpha = np.where(mask[:, t : t + 1] > 0, new, alpha)
    logZ = _logsumexp(alpha + end[None, :], -1)
    return num - logZ


def _viterbi_np(e, mask, trans, start, end):
    Bn, Tn, Ln = e.shape
    delta = start[None, :] + e[:, 0]
    ptrs = np.zeros((Tn - 1, Bn, Ln), np.int32)
    eye = np.broadcast_to(np.arange(Ln, dtype=np.int32), (Bn, Ln))
    for t in range(1, Tn):
        cand = delta[:, :, None] + trans[None]
        ptr = cand.argmax(1).astype(np.int32)
        new = cand.max(1) + e[:, t]
        mb = mask[:, t : t + 1] > 0
        delta = np.where(mb, new, delta)
        ptrs[t - 1] = np.where(mb, ptr, eye)
    y = (delta + end[None, :]).argmax(-1).astype(np.int32)
    path = [y]
    for t in range(Tn - 2, -1, -1):
        y = np.take_along_axis(ptrs[t], y[:, None], axis=1)[:, 0]
        path.append(y)
    path = np.stack(path[::-1], axis=1)
    return path * mask.astype(np.int32)


def kernel(input_embed, mask, labels, Wih_f, Whh_f, bih_f, bhh_f,
           Wih_b, Whh_b, bih_b, bhh_b, Wl, bl, trans, start, end):
    global _GRAPH, LAST_EXEC_NS
    input_embed = np.asarray(input_embed, np.float32)
    mask = np.asarray(mask, np.float32)
    labels = np.asarray(labels, np.int32)
    if _GRAPH is None:
        _GRAPH = _build_graph()
    nc = _GRAPH

    wihT = np.concatenate([np.asarray(Wih_f).T, np.asarray(Wih_b).T], axis=1)
    whhT = np.concatenate([np.asarray(Whh_f).T, np.asarray(Whh_b).T], axis=1)
    wlT = np.ascontiguousarray(np.asarray(Wl).T)
    bias_d = [np.asarray(bih_f) + np.asarray(bhh_f), np.asarray(bih_b) + np.asarray(bhh_b)]
    bias_g = np.zeros((HC, 16), np.float32)
    for m in range(16):
        d_, g_, hh = m // 8, (m // 2) % 4, m % 2
        bias_g[:, m] = bias_d[d_][g_ * H + hh * HC : g_ * H + hh * HC + HC]
    bl_in = np.asarray(bl, np.float32).reshape(L, 1)
    ident = np.eye(128, dtype=np.float32)

    in_maps = []
    for c in range(NC):
        sl = slice(c * BC, (c + 1) * BC)
        m40 = np.zeros((40, T), np.float32)
        m40[0:8] = mask[sl]
        m40[32:40] = mask[sl][:, ::-1]
        in_maps.append({
            "x": np.ascontiguousarray(input_embed[sl].reshape(BC * T, D)),
            "wihT": np.ascontiguousarray(wihT, dtype=np.float32),
            "whhT": np.ascontiguousarray(whhT, dtype=np.float32),
            "wlT": np.ascontiguousarray(wlT, dtype=np.float32),
            "bias_g": bias_g,
            "bl": bl_in,
            "mask40": m40,
            "ident": ident,
        })

    res = run_bass_kernel_spmd(nc, in_maps, list(range(NC)))
    LAST_EXEC_NS = res.exec_time_ns
    if os.environ.get("KTRACE") and LAST_EXEC_NS is None:
        import time as _time
        t0 = _time.perf_counter_ns()
        res = run_bass_kernel_spmd(nc, in_maps, list(range(NC)))
        LAST_EXEC_NS = _time.perf_counter_ns() - t0

    e = np.concatenate(
        [res.results[c]["eT"].reshape(L, T, BC).transpose(2, 1, 0) for c in range(NC)],
        axis=0,
    )  # [B, T, L]

    trans = np.asarray(trans, np.float32)
    start = np.asarray(start, np.float32)
    end = np.asarray(end, np.float32)
    llh = _crf_llh_np(e, labels, mask, trans, start, end)
    loss = np.float32(-llh.mean())
    preds = _viterbi_np(e, mask, trans, start, end)
    return loss, preds.astype(np.int32)


# revision 7
# speedup vs baseline: 1.2382x; 1.2382x over previous
import os, sys
import numpy as np

sys.path.insert(0, "/opt/trn_rl_repo")
import concourse.bass as bass
import concourse.bacc as bacc
import concourse.mybir as mybir
from concourse.tile import TileContext
from concourse.bass_utils import run_bass_kernel_spmd

F32 = mybir.dt.float32
AF = mybir.ActivationFunctionType
ALU = mybir.AluOpType

B, T, D, L, H = 64, 512, 400, 25, 200
NC = 8
BC = B // NC  # 8 sequences per core
HC = H // 2   # 100, hidden chunk (partition dim)
G4 = 4 * H    # 800 gate width per direction

_GRAPH = None
LAST_EXEC_NS = None


def _build_graph():
    nc = bacc.Bacc("TRN2", target_bir_lowering=False, debug=False, num_devices=NC)
    x = nc.dram_tensor("x", [BC * T, D], F32, kind="ExternalInput")
    wihT = nc.dram_tensor("wihT", [D, 2 * G4], F32, kind="ExternalInput")
    whhT = nc.dram_tensor("whhT", [H, 2 * G4], F32, kind="ExternalInput")
    wlT = nc.dram_tensor("wlT", [2 * H, L], F32, kind="ExternalInput")
    bias_g = nc.dram_tensor("bias_g", [HC, 16], F32, kind="ExternalInput")
    bl = nc.dram_tensor("bl", [L, 1], F32, kind="ExternalInput")
    mask40 = nc.dram_tensor("mask40", [40, T], F32, kind="ExternalInput")
    ident = nc.dram_tensor("ident", [128, 128], F32, kind="ExternalInput")
    eT = nc.dram_tensor("eT", [L, T * BC], F32, kind="ExternalOutput")
    xw = nc.dram_tensor("xw", [T, 2 * BC, G4], F32)  # scratch [t, (d,b), (g,h')]

    with TileContext(nc) as tc:
        with (
            tc.tile_pool(name="const", bufs=1) as cp,
            tc.tile_pool(name="psc", bufs=2, space="PSUM") as psc,
        ):
            ident_sb = cp.tile([128, 128], F32)
            nc.sync.dma_start(out=ident_sb[:], in_=ident[:])
            whh_sb = cp.tile([HC, 2 * 2 * G4], F32)  # chunk c at c*1600
            for c in range(2):
                nc.sync.dma_start(
                    out=whh_sb[:, c * 2 * G4 : (c + 1) * 2 * G4],
                    in_=whhT[c * HC : (c + 1) * HC, :],
                )
            mask_sb = cp.tile([40, T], F32)
            nc.sync.dma_start(out=mask_sb[:], in_=mask40[:])
            bias_sb = cp.tile([HC, 16], F32)
            nc.sync.dma_start(out=bias_sb[:], in_=bias_g[:])
            bl_sb = cp.tile([L, 1], F32)
            nc.sync.dma_start(out=bl_sb[:], in_=bl[:])
            wl_sb = cp.tile([HC, 4 * L], F32)
            for kc in range(4):
                nc.sync.dma_start(
                    out=wl_sb[:, kc * L : (kc + 1) * L],
                    in_=wlT[kc * HC : (kc + 1) * HC, :],
                )
            ztile = cp.tile([HC, BC], F32)
            nc.vector.memset(ztile[:], 0.0)
            # h history, transposed layout: col = ((d*2+c)*T + t)*BC + b
            h_hist = cp.tile([HC, 2 * 2 * T * BC], F32)
            c40 = cp.tile([40, H], F32)
            h40 = cp.tile([40, H], F32)
            nc.vector.memset(c40[:], 0.0)
            nc.vector.memset(h40[:], 0.0)

            # ---- Phase 1: transpose X -> XT [100, (k, b*T)] ----
            with (
                tc.tile_pool(name="ph1", bufs=3) as p1,
                tc.tile_pool(name="xtp", bufs=1) as xp,
                tc.tile_pool(name="ps1", bufs=4, space="PSUM") as pp1,
            ):
                XT = xp.tile([HC, 4 * BC * T], F32)
                nrow = (BC * T) // 128  # 32
                for r in range(nrow):
                    xrow = p1.tile([128, D], F32, tag="xrow")
                    nc.sync.dma_start(out=xrow[:], in_=x[r * 128 : (r + 1) * 128, :])
                    for k in range(4):
                        ps = pp1.tile([HC, 128], F32, tag="tps1")
                        nc.tensor.transpose(
                            ps[:], xrow[:, k * HC : (k + 1) * HC], ident_sb[:]
                        )
                        nc.scalar.activation(
                            XT[:, k * BC * T + r * 128 : k * BC * T + (r + 1) * 128],
                            ps[:],
                            AF.Copy,
                        )

                # ---- Phase 2: input GEMM -> xw scratch ----
                with (
                    tc.tile_pool(name="ph2", bufs=3) as p2,
                    tc.tile_pool(name="ps2", bufs=4, space="PSUM") as pp2,
                ):
                    wih_sb = []
                    for k in range(4):
                        wt = xp.tile([HC, 2 * G4], F32, tag=f"wih{k}")
                        nc.sync.dma_start(
                            out=wt[:], in_=wihT[k * HC : (k + 1) * HC, :]
                        )
                        wih_sb.append(wt)
                    xw_r = xw[:].rearrange("t r c -> r c t")  # [(d,b), (g,h'), t]
                    for m in range(16):  # (d, g, hh)
                        d_, g_, hh = m // 8, (m // 2) % 4, m % 2
                        for b8 in range(BC):
                            ps = pp2.tile([HC, T], F32, tag="ps2t")
                            for k in range(4):
                                nc.tensor.matmul(
                                    ps[:],
                                    wih_sb[k][:, m * HC : (m + 1) * HC],
                                    XT[:, k * BC * T + b8 * T : k * BC * T + (b8 + 1) * T],
                                    start=(k == 0),
                                    stop=(k == 3),
                                )
                            tmp = p2.tile([HC, T], F32, tag="tmp2")
                            nc.scalar.activation(
                                tmp[:], ps[:], AF.Identity, bias=bias_sb[:, m : m + 1]
                            )
                            col0 = g_ * H + hh * HC
                            nc.sync.dma_start(
                                out=xw_r[d_ * BC + b8, col0 : col0 + HC, :],
                                in_=tmp[:],
                            )

            # ---- Phase 3: BiLSTM scan ----
            with (
                tc.tile_pool(name="ph3", bufs=3) as p3,
                tc.tile_pool(name="ps3", bufs=2, space="PSUM") as pp3,
            ):
                for t in range(T):
                    tb = T - 1 - t  # backward time index
                    # xw tile: rows 0-8 = fwd (d=0) at t, rows 32-40 = bwd at tb
                    xwt = p3.tile([40, G4], F32, tag="xwt")
                    nc.sync.dma_start(out=xwt[0:8, :], in_=xw[t, 0:BC, :])
                    nc.sync.dma_start(out=xwt[32:40, :], in_=xw[tb, BC : 2 * BC, :])
                    gps0 = pp3.tile([40, 400], F32, tag="gp0")
                    gps1 = pp3.tile([40, 400], F32, tag="gp1")
                    for d_ in range(2):
                        tt = t if d_ == 0 else tb
                        tprev = tt - 1 if d_ == 0 else tt + 1
                        for nh in range(2):
                            dst = (gps0 if nh == 0 else gps1)[
                                d_ * 32 : d_ * 32 + 8, :
                            ]
                            # seed psum with xw via identity matmul (off critical path)
                            nc.tensor.matmul(
                                dst,
                                ident_sb[d_ * 32 : d_ * 32 + 8, d_ * 32 : d_ * 32 + 8],
                                xwt[d_ * 32 : d_ * 32 + 8, nh * 400 : (nh + 1) * 400],
                                start=True,
                                stop=False,
                            )
                            for c in range(2):
                                if t == 0:
                                    lhsT = ztile[:]
                                else:
                                    off = ((d_ * 2 + c) * T + tprev) * BC
                                    lhsT = h_hist[:, off : off + BC]
                                rhs = whh_sb[
                                    :,
                                    c * 2 * G4 + d_ * G4 + nh * 400 : c * 2 * G4
                                    + d_ * G4
                                    + (nh + 1) * 400,
                                ]
                                nc.tensor.matmul(
                                    dst, lhsT, rhs, start=False, stop=(c == 1)
                                )
                    s0 = p3.tile([40, 400], F32, tag="s0")
                    th = p3.tile([40, 400], F32, tag="th")
                    nc.scalar.activation(s0[:], gps0[:], AF.Sigmoid)  # i | f
                    nc.scalar.activation(th[:, 0:H], gps1[:, 0:H], AF.Tanh)  # g~
                    nc.scalar.activation(th[:, H:400], gps1[:, H:400], AF.Sigmoid)  # o
                    t1 = p3.tile([40, H], F32, tag="t1")
                    t2 = p3.tile([40, H], F32, tag="t2")
                    cn = p3.tile([40, H], F32, tag="cn")
                    nc.vector.tensor_mul(t1[:], s0[:, 0:H], th[:, 0:H])
                    nc.vector.tensor_mul(t2[:], s0[:, H:400], c40[:])
                    nc.vector.tensor_add(cn[:], t1[:], t2[:])
                    cd = p3.tile([40, H], F32, tag="cd")
                    nc.vector.tensor_sub(cd[:], cn[:], c40[:])
                    nc.vector.scalar_tensor_tensor(
                        c40[:], cd[:], mask_sb[:, t : t + 1], c40[:],
                        op0=ALU.mult, op1=ALU.add,
                    )
                    thc = p3.tile([40, H], F32, tag="thc")
                    nc.scalar.activation(thc[:], c40[:], AF.Tanh)
                    hn = p3.tile([40, H], F32, tag="hn")
                    nc.vector.tensor_mul(hn[:], th[:, H:400], thc[:])
                    hd = p3.tile([40, H], F32, tag="hd")
                    nc.vector.tensor_sub(hd[:], hn[:], h40[:])
                    nc.vector.scalar_tensor_tensor(
                        h40[:], hd[:], mask_sb[:, t : t + 1], h40[:],
                        op0=ALU.mult, op1=ALU.add,
                    )
                    for d_ in range(2):
                        tt = t if d_ == 0 else tb
                        for c in range(2):
                            tps = pp3.tile([HC, BC], F32, tag="tp3")
                            nc.tensor.transpose(
                                tps[:],
                                h40[d_ * 32 : d_ * 32 + 8, c * HC : (c + 1) * HC],
                                ident_sb[d_ * 32 : d_ * 32 + 8, d_ * 32 : d_ * 32 + 8],
                            )
                            off = ((d_ * 2 + c) * T + tt) * BC
                            nc.scalar.activation(
                                h_hist[:, off : off + BC], tps[:], AF.Copy
                            )

            # ---- Phase 4: emissions GEMM -> eT ----
            with (
                tc.tile_pool(name="ph4", bufs=3) as p4,
                tc.tile_pool(name="ps4", bufs=4, space="PSUM") as pp4,
            ):
                for n in range(8):
                    ps = pp4.tile([L, T], F32, tag="ps4t")
                    for kc in range(4):
                        rhs = h_hist[:, kc * T * BC + n * T : kc * T * BC + (n + 1) * T]
                        nc.tensor.matmul(
                            ps[:],
                            wl_sb[:, kc * L : (kc + 1) * L],
                            rhs,
                            start=(kc == 0),
                            stop=(kc == 3),
                        )
                    et = p4.tile([L, T], F32, tag="et4")
                    nc.scalar.activation(et[:], ps[:], AF.Identity, bias=bl_sb[:])
                    nc.sync.dma_start(out=eT[:, n * T : (n + 1) * T], in_=et[:])

    nc.compile()
    return nc


def _logsumexp(a, axis):
    m = a.max(axis=axis, keepdims=True)
    return (m + np.log(np.exp(a - m).sum(axis=axis, keepdims=True))).squeeze(axis)


def _crf_llh_np(e, labels, mask, trans, start, end):
    em = np.take_along_axis(e, labels[..., None], axis=-1)[..., 0]
    tr = trans[labels[:, :-1], labels[:, 1:]]
    last = (mask.sum(1) - 1.0).astype(np.int32)
    y_last = np.take_along_axis(labels, last[:, None], axis=1)[:, 0]
    num = (
        start[labels[:, 0]]
        + em[:, 0]
        + (mask[:, 1:] * (em[:, 1:] + tr)).sum(1)
        + end[y_last]
    )
    alpha = start[None, :] + e[:, 0]
    for t in range(1, e.shape[1]):
        cand = alpha[:, :, None] + trans[None]
        m = cand.max(1)
        new = m + np.log(np.exp(cand - m[:, None, :]).sum(1)) + e[:, t]
        alpha = np.where(mask[:, t : t + 1] > 0, new, alpha)
    logZ = _logsumexp(alpha + end[None, :], -1)
    return num - logZ


def _viterbi_np(e, mask, trans, start, end):
    Bn, Tn, Ln = e.shape
    delta = start[None, :] + e[:, 0]
    ptrs = np.zeros((Tn - 1, Bn, Ln), np.int32)
    eye = np.broadcast_to(np.arange(Ln, dtype=np.int32), (Bn, Ln))
    for t in range(1, Tn):
        cand = delta[:, :, None] + trans[None]
        ptr = cand.argmax(1).astype(np.int32)
        new = cand.max(1) + e[:, t]
        mb = mask[:, t : t + 1] > 0
        delta = np.where(mb, new, delta)
        ptrs[t - 1] = np.where(mb, ptr, eye)
    y = (delta + end[None, :]).argmax(-1).astype(np.int32)
    path = [y]
    for t in range(Tn - 2, -1, -1):
        y = np.take_along_axis(ptrs[t], y[:, None], axis=1)[:, 0]
        path.append(y)
    path = np.stack(path[::-1], axis=1)
    return path * mask.astype(np.int32)


def kernel(input_embed, mask, labels, Wih_f, Whh_f, bih_f, bhh_f,
           Wih_b, Whh_b, bih_b, bhh_b, Wl, bl, trans, start, end):
    global _GRAPH, LAST_EXEC_NS
    input_embed = np.asarray(input_embed, np.float32)
    mask = np.asarray(mask, np.float32)
    labels = np.asarray(labels, np.int32)
    if _GRAPH is None:
        _GRAPH = _build_graph()
    nc = _GRAPH

    wihT = np.concatenate([np.asarray(Wih_f).T, np.asarray(Wih_b).T], axis=1)
    whhT = np.concatenate([np.asarray(Whh_f).T, np.asarray(Whh_b).T], axis=1)
    wlT = np.ascontiguousarray(np.asarray(Wl).T)
    bias_d = [np.asarray(bih_f) + np.asarray(bhh_f), np.asarray(bih_b) + np.asarray(bhh_b)]
    bias_g = np.zeros((HC, 16), np.float32)
    for m in range(16):
        d_, g_, hh = m // 8, (m // 2) % 4, m % 2
        bias_g[:, m] = bias_d[d_][g_ * H + hh * HC : g_ * H + hh * HC + HC]
    bl_in = np.asarray(bl, np.float32).reshape(L, 1)
    ident = np.eye(128, dtype=np.float32)

    in_maps = []
    for c in range(NC):
        sl = slice(c * BC, (c + 1) * BC)
        m40 = np.zeros((40, T), np.float32)
        m40[0:8] = mask[sl]
        m40[32:40] = mask[sl][:, ::-1]
        in_maps.append({
            "x": np.ascontiguousarray(input_embed[sl].reshape(BC * T, D)),
            "wihT": np.ascontiguousarray(wihT, dtype=np.float32),
            "whhT": np.ascontiguousarray(whhT, dtype=np.float32),
            "wlT": np.ascontiguousarray(wlT, dtype=np.float32),
            "bias_g": bias_g,
            "bl": bl_in,
            "mask40": m40,
            "ident": ident,
        })

    res = run_bass_kernel_spmd(nc, in_maps, list(range(NC)))
    LAST_EXEC_NS = res.exec_time_ns
    if os.environ.get("KTRACE") and LAST_EXEC_NS is None:
        import time as _time
        t0 = _time.perf_counter_ns()
        res = run_bass_kernel_spmd(nc, in_maps, list(range(NC)))
        LAST_EXEC_NS = _time.perf_counter_ns() - t0

    e = np.concatenate(
        [res.results[c]["eT"].reshape(L, T, BC).transpose(2, 1, 0) for c in range(NC)],
        axis=0,
    )  # [B, T, L]

    trans = np.asarray(trans, np.float32)
    start = np.asarray(start, np.float32)
    end = np.asarray(end, np.float32)
    llh = _crf_llh_np(e, labels, mask, trans, start, end)
    loss = np.float32(-llh.mean())
    preds = _viterbi_np(e, mask, trans, start, end)
    return loss, preds.astype(np.int32)


# revision 10
# speedup vs baseline: 5.9659x; 4.8181x over previous
import os, sys
import numpy as np

sys.path.insert(0, "/opt/trn_rl_repo")
import concourse.bass as bass
import concourse.bacc as bacc
import concourse.mybir as mybir
from concourse.tile import TileContext
from concourse.bass_utils import run_bass_kernel_spmd

F32 = mybir.dt.float32
AF = mybir.ActivationFunctionType
ALU = mybir.AluOpType

B, T, D, L, H = 64, 512, 400, 25, 200
NC = 8
BC = B // NC  # 8 sequences per core
HC = H // 2   # 100, hidden chunk (partition dim)
G4 = 4 * H    # 800 gate width per direction

_GRAPH = None
LAST_EXEC_NS = None


def _build_graph():
    nc = bacc.Bacc("TRN2", target_bir_lowering=False, debug=False, num_devices=NC)
    x = nc.dram_tensor("x", [BC * T, D], F32, kind="ExternalInput")
    wihT = nc.dram_tensor("wihT", [D, 2 * G4], F32, kind="ExternalInput")
    whhT = nc.dram_tensor("whhT", [H, 2 * G4], F32, kind="ExternalInput")
    wlT = nc.dram_tensor("wlT", [2 * H, L], F32, kind="ExternalInput")
    bias_g = nc.dram_tensor("bias_g", [HC, 16], F32, kind="ExternalInput")
    bl = nc.dram_tensor("bl", [L, 1], F32, kind="ExternalInput")
    mask40 = nc.dram_tensor("mask40", [40, T], F32, kind="ExternalInput")
    ident = nc.dram_tensor("ident", [128, 128], F32, kind="ExternalInput")
    eT = nc.dram_tensor("eT", [L, T * BC], F32, kind="ExternalOutput")
    xw = nc.dram_tensor("xw", [T, 2 * BC, G4], F32)  # scratch [t, (d,b), (g,h')]

    with TileContext(nc) as tc:
        with (
            tc.tile_pool(name="const", bufs=1) as cp,
            tc.tile_pool(name="psc", bufs=2, space="PSUM") as psc,
        ):
            ident_sb = cp.tile([128, 128], F32)
            nc.sync.dma_start(out=ident_sb[:], in_=ident[:])
            whh_sb = cp.tile([HC, 2 * 2 * G4], F32)  # chunk c at c*1600
            for c in range(2):
                nc.sync.dma_start(
                    out=whh_sb[:, c * 2 * G4 : (c + 1) * 2 * G4],
                    in_=whhT[c * HC : (c + 1) * HC, :],
                )
            mask_sb = cp.tile([40, T], F32)
            nc.sync.dma_start(out=mask_sb[:], in_=mask40[:])
            bias_sb = cp.tile([HC, 16], F32)
            nc.sync.dma_start(out=bias_sb[:], in_=bias_g[:])
            bl_sb = cp.tile([L, 1], F32)
            nc.sync.dma_start(out=bl_sb[:], in_=bl[:])
            wl_sb = cp.tile([HC, 4 * L], F32)
            for kc in range(4):
                nc.sync.dma_start(
                    out=wl_sb[:, kc * L : (kc + 1) * L],
                    in_=wlT[kc * HC : (kc + 1) * HC, :],
                )
            ztile = cp.tile([HC, BC], F32)
            nc.vector.memset(ztile[:], 0.0)
            # h history, transposed layout: col = ((d*2+c)*T + t)*BC + b
            h_hist = cp.tile([HC, 2 * 2 * T * BC], F32)
            c40 = cp.tile([40, H], F32)
            h40 = cp.tile([40, H], F32)
            nc.vector.memset(c40[:], 0.0)
            nc.vector.memset(h40[:], 0.0)

            # ---- Phase 1: transpose X -> XT [100, (k, b*T)] ----
            with (
                tc.tile_pool(name="ph1", bufs=3) as p1,
                tc.tile_pool(name="xtp", bufs=1) as xp,
                tc.tile_pool(name="ps1", bufs=4, space="PSUM") as pp1,
            ):
                XT = xp.tile([HC, 4 * BC * T], F32)
                nrow = (BC * T) // 128  # 32
                for r in range(nrow):
                    xrow = p1.tile([128, D], F32, tag="xrow")
                    nc.sync.dma_start(out=xrow[:], in_=x[r * 128 : (r + 1) * 128, :])
                    for k in range(4):
                        ps = pp1.tile([HC, 128], F32, tag="tps1")
                        nc.tensor.transpose(
                            ps[:], xrow[:, k * HC : (k + 1) * HC], ident_sb[:]
                        )
                        nc.scalar.activation(
                            XT[:, k * BC * T + r * 128 : k * BC * T + (r + 1) * 128],
                            ps[:],
                            AF.Copy,
                        )

                # ---- Phase 2: input GEMM -> xw scratch ----
                with (
                    tc.tile_pool(name="ph2", bufs=3) as p2,
                    tc.tile_pool(name="ps2", bufs=4, space="PSUM") as pp2,
                ):
                    wih_sb = []
                    for k in range(4):
                        wt = xp.tile([HC, 2 * G4], F32, tag=f"wih{k}")
                        nc.sync.dma_start(
                            out=wt[:], in_=wihT[k * HC : (k + 1) * HC, :]
                        )
                        wih_sb.append(wt)
                    xw_r = xw[:].rearrange("t r c -> r c t")  # [(d,b), (g,h'), t]
                    for m in range(16):  # (d, g, hh)
                        d_, g_, hh = m // 8, (m // 2) % 4, m % 2
                        for b8 in range(BC):
                            ps = pp2.tile([HC, T], F32, tag="ps2t")
                            for k in range(4):
                                nc.tensor.matmul(
                                    ps[:],
                                    wih_sb[k][:, m * HC : (m + 1) * HC],
                                    XT[:, k * BC * T + b8 * T : k * BC * T + (b8 + 1) * T],
                                    start=(k == 0),
                                    stop=(k == 3),
                                )
                            tmp = p2.tile([HC, T], F32, tag="tmp2")
                            nc.scalar.activation(
                                tmp[:], ps[:], AF.Identity, bias=bias_sb[:, m : m + 1]
                            )
                            col0 = g_ * H + hh * HC
                            nc.sync.dma_start(
                                out=xw_r[d_ * BC + b8, col0 : col0 + HC, :],
                                in_=tmp[:],
                            )

            # ---- Phase 3: BiLSTM scan ----
            with (
                tc.tile_pool(name="ph3", bufs=3) as p3,
                tc.tile_pool(name="ps3", bufs=2, space="PSUM") as pp3,
            ):
                for t in range(T):
                    tb = T - 1 - t  # backward time index
                    # xw tile: rows 0-8 = fwd (d=0) at t, rows 32-40 = bwd at tb
                    xwt = p3.tile([40, G4], F32, tag="xwt")
                    nc.sync.dma_start(out=xwt[0:8, :], in_=xw[t, 0:BC, :])
                    nc.sync.dma_start(out=xwt[32:40, :], in_=xw[tb, BC : 2 * BC, :])
                    gps0 = pp3.tile([40, 400], F32, tag="gp0")
                    gps1 = pp3.tile([40, 400], F32, tag="gp1")
                    for d_ in range(2):
                        tt = t if d_ == 0 else tb
                        tprev = tt - 1 if d_ == 0 else tt + 1
                        for nh in range(2):
                            dst = (gps0 if nh == 0 else gps1)[
                                d_ * 32 : d_ * 32 + 8, :
                            ]
                            # seed psum with xw via identity matmul (off critical path)
                            nc.tensor.matmul(
                                dst,
                                ident_sb[d_ * 32 : d_ * 32 + 8, d_ * 32 : d_ * 32 + 8],
                                xwt[d_ * 32 : d_ * 32 + 8, nh * 400 : (nh + 1) * 400],
                                start=True,
                                stop=False,
                            )
                            for c in range(2):
                                if t == 0:
                                    lhsT = ztile[:]
                                else:
                                    off = ((d_ * 2 + c) * T + tprev) * BC
                                    lhsT = h_hist[:, off : off + BC]
                                rhs = whh_sb[
                                    :,
                                    c * 2 * G4 + d_ * G4 + nh * 400 : c * 2 * G4
                                    + d_ * G4
                                    + (nh + 1) * 400,
                                ]
                                nc.tensor.matmul(
                                    dst, lhsT, rhs, start=False, stop=(c == 1)
                                )
                    s0 = p3.tile([40, 400], F32, tag="s0")
                    th = p3.tile([40, 400], F32, tag="th")
                    nc.scalar.activation(s0[:], gps0[:], AF.Sigmoid)  # i | f
                    nc.scalar.activation(th[:, 0:H], gps1[:, 0:H], AF.Tanh)  # g~
                    nc.scalar.activation(th[:, H:400], gps1[:, H:400], AF.Sigmoid)  # o
                    t1 = p3.tile([40, H], F32, tag="t1")
                    t2 = p3.tile([40, H], F32, tag="t2")
                    cn = p3.tile([40, H], F32, tag="cn")
                    nc.vector.tensor_mul(t1[:], s0[:, 0:H], th[:, 0:H])
                    nc.vector.tensor_mul(t2[:], s0[:, H:400], c40[:])
                    nc.vector.tensor_add(cn[:], t1[:], t2[:])
                    cd = p3.tile([40, H], F32, tag="cd")
                    nc.vector.tensor_sub(cd[:], cn[:], c40[:])
                    nc.vector.scalar_tensor_tensor(
                        c40[:], cd[:], mask_sb[:, t : t + 1], c40[:],
                        op0=ALU.mult, op1=ALU.add,
                    )
                    thc = p3.tile([40, H], F32, tag="thc")
                    nc.scalar.activation(thc[:], c40[:], AF.Tanh)
                    hn = p3.tile([40, H], F32, tag="hn")
                    nc.vector.tensor_mul(hn[:], th[:, H:400], thc[:])
                    hd = p3.tile([40, H], F32, tag="hd")
                    nc.vector.tensor_sub(hd[:], hn[:], h40[:])
                    nc.vector.scalar_tensor_tensor(
                        h40[:], hd[:], mask_sb[:, t : t + 1], h40[:],
                        op0=ALU.mult, op1=ALU.add,
                    )
                    for d_ in range(2):
                        tt = t if d_ == 0 else tb
                        for c in range(2):
                            tps = pp3.tile([HC, BC], F32, tag="tp3")
                            nc.tensor.transpose(
                                tps[:],
                                h40[d_ * 32 : d_ * 32 + 8, c * HC : (c + 1) * HC],
                                ident_sb[d_ * 32 : d_ * 32 + 8, d_ * 32 : d_ * 32 + 8],
                            )
                            off = ((d_ * 2 + c) * T + tt) * BC
                            nc.scalar.activation(
                                h_hist[:, off : off + BC], tps[:], AF.Copy
                            )

            # ---- Phase 4: emissions GEMM -> eT ----
            with (
                tc.tile_pool(name="ph4", bufs=3) as p4,
                tc.tile_pool(name="ps4", bufs=4, space="PSUM") as pp4,
            ):
                for n in range(8):
                    ps = pp4.tile([L, T], F32, tag="ps4t")
                    for kc in range(4):
                        rhs = h_hist[:, kc * T * BC + n * T : kc * T * BC + (n + 1) * T]
                        nc.tensor.matmul(
                            ps[:],
                            wl_sb[:, kc * L : (kc + 1) * L],
                            rhs,
                            start=(kc == 0),
                            stop=(kc == 3),
                        )
                    et = p4.tile([L, T], F32, tag="et4")
                    nc.scalar.activation(et[:], ps[:], AF.Identity, bias=bl_sb[:])
                    nc.sync.dma_start(out=eT[:, n * T : (n + 1) * T], in_=et[:])

    nc.compile()
    return nc


_EXEC = None


def _get_exec(nc):
    """Build the jitted SPMD executable once; bass2jax rebuilds it per call,
    costing seconds of retrace + XLA rebuild on every kernel() invocation."""
    global _EXEC
    if _EXEC is not None:
        return _EXEC
    import jax
    from jax.sharding import Mesh, PartitionSpec
    from jax.experimental.shard_map import shard_map
    from concourse import bass2jax as b2j

    b2j.install_neuronx_cc_hook()
    partition_name = nc.partition_id_tensor.name if nc.partition_id_tensor else None
    in_names, out_names, out_avals, zero_outs = [], [], [], []
    for alloc in nc.m.functions[0].allocations:
        if not isinstance(alloc, mybir.MemoryLocationSet):
            continue
        name = alloc.memorylocations[0].name
        if alloc.kind == "ExternalInput":
            if name != partition_name:
                in_names.append(name)
        elif alloc.kind == "ExternalOutput":
            shape = tuple(alloc.tensor_shape)
            dtype = mybir.dt.np(alloc.dtype)
            out_names.append(name)
            out_avals.append(jax.core.ShapedArray(shape, dtype))
            zero_outs.append(np.zeros(shape, dtype))
    n_params = len(in_names)
    n_outs = len(out_avals)
    in_names_full = list(in_names) + list(out_names)
    if partition_name is not None:
        in_names_full.append(partition_name)
    donate = tuple(range(n_params, n_params + n_outs))

    def _body(*args):
        operands = list(args)
        if partition_name is not None:
            operands.append(b2j.partition_id_tensor())
        outs = b2j._bass_exec_p.bind(
            *operands,
            out_avals=tuple(out_avals),
            in_names=tuple(in_names_full),
            out_names=tuple(out_names),
            lowering_input_output_aliases=(),
            sim_require_finite=True,
            sim_require_nnan=True,
            nc=nc,
        )
        return tuple(outs)

    devices = jax.devices()[:NC]
    mesh = Mesh(np.asarray(devices), ("core",))
    in_specs = (PartitionSpec("core"),) * (n_params + n_outs)
    out_specs = (PartitionSpec("core"),) * n_outs
    sharded = jax.jit(
        shard_map(
            _body, mesh=mesh, in_specs=in_specs, out_specs=out_specs, check_rep=False
        ),
        donate_argnums=donate,
        keep_unused=True,
    )
    _EXEC = (sharded, in_names, out_names, out_avals, zero_outs)
    return _EXEC


def _run_spmd(nc, in_maps):
    sharded, in_names, out_names, out_avals, zero_outs = _get_exec(nc)
    per_core = [[np.asarray(m[n]) for n in in_names] for m in in_maps]
    concat_in = [
        np.concatenate([per_core[c][i] for c in range(NC)], axis=0)
        for i in range(len(in_names))
    ]
    concat_zeros = [
        np.zeros((NC * z.shape[0], *z.shape[1:]), z.dtype) for z in zero_outs
    ]
    out_arrs = sharded(*concat_in, *concat_zeros)
    return [
        {
            name: np.asarray(out_arrs[i]).reshape(NC, *out_avals[i].shape)[c]
            for i, name in enumerate(out_names)
        }
        for c in range(NC)
    ]


def _logsumexp(a, axis):
    m = a.max(axis=axis, keepdims=True)
    return (m + np.log(np.exp(a - m).sum(axis=axis, keepdims=True))).squeeze(axis)


def _crf_llh_np(e, labels, mask, trans, start, end):
    em = np.take_along_axis(e, labels[..., None], axis=-1)[..., 0]
    tr = trans[labels[:, :-1], labels[:, 1:]]
    last = (mask.sum(1) - 1.0).astype(np.int32)
    y_last = np.take_along_axis(labels, last[:, None], axis=1)[:, 0]
    num = (
        start[labels[:, 0]]
        + em[:, 0]
        + (mask[:, 1:] * (em[:, 1:] + tr)).sum(1)
        + end[y_last]
    )
    alpha = start[None, :] + e[:, 0]
    for t in range(1, e.shape[1]):
        cand = alpha[:, :, None] + trans[None]
        m = cand.max(1)
        new = m + np.log(np.exp(cand - m[:, None, :]).sum(1)) + e[:, t]
        alpha = np.where(mask[:, t : t + 1] > 0, new, alpha)
    logZ = _logsumexp(alpha + end[None, :], -1)
    return num - logZ


def _viterbi_np(e, mask, trans, start, end):
    Bn, Tn, Ln = e.shape
    delta = start[None, :] + e[:, 0]
    ptrs = np.zeros((Tn - 1, Bn, Ln), np.int32)
    eye = np.broadcast_to(np.arange(Ln, dtype=np.int32), (Bn, Ln))
    for t in range(1, Tn):
        cand = delta[:, :, None] + trans[None]
        ptr = cand.argmax(1).astype(np.int32)
        new = cand.max(1) + e[:, t]
        mb = mask[:, t : t + 1] > 0
        delta = np.where(mb, new, delta)
        ptrs[t - 1] = np.where(mb, ptr, eye)
    y = (delta + end[None, :]).argmax(-1).astype(np.int32)
    path = [y]
    for t in range(Tn - 2, -1, -1):
        y = np.take_along_axis(ptrs[t], y[:, None], axis=1)[:, 0]
        path.append(y)
    path = np.stack(path[::-1], axis=1)
    return path * mask.astype(np.int32)


def kernel(input_embed, mask, labels, Wih_f, Whh_f, bih_f, bhh_f,
           Wih_b, Whh_b, bih_b, bhh_b, Wl, bl, trans, start, end):
    global _GRAPH, LAST_EXEC_NS
    input_embed = np.asarray(input_embed, np.float32)
    mask = np.asarray(mask, np.float32)
    labels = np.asarray(labels, np.int32)
    if _GRAPH is None:
        _GRAPH = _build_graph()
    nc = _GRAPH

    wihT = np.concatenate([np.asarray(Wih_f).T, np.asarray(Wih_b).T], axis=1)
    whhT = np.concatenate([np.asarray(Whh_f).T, np.asarray(Whh_b).T], axis=1)
    wlT = np.ascontiguousarray(np.asarray(Wl).T)
    bias_d = [np.asarray(bih_f) + np.asarray(bhh_f), np.asarray(bih_b) + np.asarray(bhh_b)]
    bias_g = np.zeros((HC, 16), np.float32)
    for m in range(16):
        d_, g_, hh = m // 8, (m // 2) % 4, m % 2
        bias_g[:, m] = bias_d[d_][g_ * H + hh * HC : g_ * H + hh * HC + HC]
    bl_in = np.asarray(bl, np.float32).reshape(L, 1)
    ident = np.eye(128, dtype=np.float32)

    in_maps = []
    for c in range(NC):
        sl = slice(c * BC, (c + 1) * BC)
        m40 = np.zeros((40, T), np.float32)
        m40[0:8] = mask[sl]
        m40[32:40] = mask[sl][:, ::-1]
        in_maps.append({
            "x": np.ascontiguousarray(input_embed[sl].reshape(BC * T, D)),
            "wihT": np.ascontiguousarray(wihT, dtype=np.float32),
            "whhT": np.ascontiguousarray(whhT, dtype=np.float32),
            "wlT": np.ascontiguousarray(wlT, dtype=np.float32),
            "bias_g": bias_g,
            "bl": bl_in,
            "mask40": m40,
            "ident": ident,
        })

    outs = _run_spmd(nc, in_maps)
    if os.environ.get("KTRACE"):
        import time as _time
        t0 = _time.perf_counter_ns()
        outs = _run_spmd(nc, in_maps)
        LAST_EXEC_NS = _time.perf_counter_ns() - t0

    e = np.concatenate(
        [outs[c]["eT"].reshape(L, T, BC).transpose(2, 1, 0) for c in range(NC)],
        axis=0,
    )  # [B, T, L]

    trans = np.asarray(trans, np.float32)
    start = np.asarray(start, np.float32)
    end = np.asarray(end, np.float32)
    llh = _crf_llh_np(e, labels, mask, trans, start, end)
    loss = np.float32(-llh.mean())
    preds = _viterbi_np(e, mask, trans, start, end)
    return loss, preds.astype(np.int32)
